# revision 54
# baseline (speedup 1.0000x reference)
"""AlloLayer forward on 8 TRN2 NeuronCores.

Math: reference computes
    lp   = log_softmax(hs, -1)                      # [B,T,C]
    ex   = exp(lp[..., phone_lab] + alloW)          # [B,T,A]
    sq   = scatter_add(ex, phoneme_lab)             # [B,T,P]
    red  = (sq.sum(-1) - 1) / P
    out  = log(sq - red)

The gather+exp+scatter collapses into a matmul: with
    M[c,p] = sum_{a: phone_lab[a]==c, phoneme_lab[a]==p} exp(alloW[a])
we have sq = softmax(hs) @ M.  The redistribution bias is linear, so it is
baked into M on the host:
    M'[c,p] = M[c,p] + (1 - rowsum(M)[c]) / P
    M'aug   = [M' | ones]                           # [C, 257]
Then per 128-row block (production pipeline, build_graph_t2,
epi="fastlog"):
    U' = exp(X) @ M'aug     (ACT exp -> bf16; PE, f32 PSUM accumulate; U'
                             cols [0:P] hold sq + (s-w)/P, col P = s)
    f_s = f32(i32(s))       (tiny strided DVE convert, one per ugrp blocks;
                             block pairs share one 2-bank PSUM tile so the
                             group's s columns sit on a regular stride)
    out = (i32(U') - f_s) * ln2/2^23               (ONE DVE tensor_scalar)
The last line is the bitcast-log identity ln(u/s) = (i32(u)-i32(s))*k +
ln2*(eps_u-eps_s): the 127 exponent bias and the mean of the mantissa
sawtooth eps in [0,0.0861] cancel in the subtraction.  Residual error
<=0.06 ln-units (rel_l2 4.2e-3, max elementwise 1.7e-2 vs the 2e-2 gate) —
and the whole Ln+normalize epilogue leaves ACT, whose exp stream was the
serial bottleneck.  epi="fastlog2" (exact ACT ln(s) + centered sawtooth,
maxrel 1.28e-2) and mm_fp8 (fp8e4 DoubleRow matmuls, 2 MMs/block, ~1.4x PE
but maxrel 2.1e-2+) are implemented but not enabled: the first costs ~1.5us
of per-block latency, the second eats the elementwise-error margin.

Data-parallel over B*T rows: 16384 rows -> 2048 rows per core, no cross-core
communication; output gathered + upcast on host.  Host hands each core its
shard in a TILED layout ([NT, 128, KCH*rs] bf16, per-partition contiguous)
so the contraction dim sits on SBUF partitions (no on-chip transpose) and
every DMA moves dense 4KB-per-partition runs; output returns group-tiled
([NGRP, 128, out_blocks*P] bf16) and is unscrambled on the host.
M'aug/labels are tiny and precomputed on host.  In-DMAs ride the SP HWDGE
ring, out-DMAs the ACT HWDGE ring (out_eng="scalar") so neither queues
behind the other; warm_mms dummy matmuls keep the PE HAM clock-gate at
8/8 through the pipeline fill; the bench loop uses the staggered-reset
back-edge AND unroll=8 (eight kernel bodies per For_i iteration emitted
back-to-back with shared tile pools) so consecutive executions
software-pipeline instead of draining at an all-engine barrier —
unroll 1/2/4/8 measured 20.4/17.1/14.9/14.3 us in one interleaved batch.

Measured (two-loop-length wall differencing, 64Ki vs 128Ki total on-device
kernel executions, 8 cores): 11.3 us steady-state vs the 23.1 us staged
baseline (same-session re-measure of that baseline: 26.4 us), rel_l2
4.185e-3.  TimelineSim span analysis: per body PE busy ~9.4 us is the
largest stream (64 MM x ~107-124 ns); the residual ~4 us/body beyond the
engine max is per-body cross-engine latency (PE/ACT gaps ~3.5-4 us) that
deeper tile-pool buffering, per-body warm MMs, and in-DMA ring-splitting
did NOT close (all tested).  Single-shot latency terms: startup ~4.9 us
(DMA first-byte + completion-sem latency + exp0), tail ~5.8 us, of which
~1.6 us is out-DMA completion latency.
"""

import os
import numpy as np

import concourse.bass as bass
import concourse.tile as tile
from concourse import bacc, mybir
from concourse import bass_utils

F32 = mybir.dt.float32
F32R = mybir.dt.float32r
BF16 = mybir.dt.bfloat16

N_CORES = 8
B, T, C, A, P = 16, 1024, 512, 4096, 256
ROWS = B * T                      # 16384
R_PER_CORE = ROWS // N_CORES      # 2048
NAUG = P + 3                      # 259: [M | (M@1)/P | ones | ones/P]
BLK = 128
NBLK = R_PER_CORE // BLK          # 16
SUPER = 4                         # row-blocks per DMA superblock
NSUPER = NBLK // SUPER            # 4
KCH = C // BLK                    # 4 contraction chunks


def _pin_act_table(arch):
    """Make natural_log_exp_and_others the only table-set advertising Exp/Ln.

    The compiled NEFF then keeps one resident ACT table set for the whole
    kernel instead of reloading (~1.3us each) on every Exp<->Ln alternation.
    Temporarily mutates the functools-cached dict (set indices unchanged; the
    real HW set genuinely contains both functions, so execution is
    unaffected); returns a restore() closure to undo it after compile.
    """
    from concourse import hw_specs

    tabs = hw_specs.get_activation_tables(arch)
    both = "natural_log_exp_and_others"
    assert both in tabs
    af = mybir.ActivationFunctionType
    assert af.Exp in tabs[both] and af.Ln in tabs[both]
    removed = []
    for name, fns in tabs.items():
        if name != both:
            for f in (af.Exp, af.Ln):
                if f in fns:
                    fns.discard(f)
                    removed.append((fns, f))

    def restore():
        for fns, f in removed:
            fns.add(f)

    return restore


def build_graph(x_bufs=4, e_bufs=3, o_bufs=3, xt_bufs=3, u_bufs=2, out_batch=2,
                bench_iters=0, dma_only=False, compute_only=False,
                out_on_sync=False, bf16_t=False, cast_eng="vector"):
    from contextlib import ExitStack, nullcontext

    nc = bacc.Bacc("TRN2", target_bir_lowering=False, debug=False, num_devices=1)
    _restore_tabs = _pin_act_table(nc.m.arch)
    x_ap = nc.dram_tensor("x", [R_PER_CORE, C], F32, kind="ExternalInput").ap()
    maug_ap = nc.dram_tensor("maug", [KCH, BLK, NAUG], F32, kind="ExternalInput").ap()
    ident_ap = nc.dram_tensor("ident", [BLK, BLK], F32, kind="ExternalInput").ap()
    out_ap = nc.dram_tensor("out", [R_PER_CORE, P], F32, kind="ExternalOutput").ap()

    # Pair schedule: 1-block pairs at both ends — fast pipeline fill at the
    # start, short drain chain at the end.
    PAIRS = [1, 1] + [2] * ((NBLK - 4) // 2) + [1, 1]
    assert sum(PAIRS) == NBLK

    with tile.TileContext(nc) as tc, ExitStack() as ctx:
        const_pool = ctx.enter_context(tc.tile_pool(name="const", bufs=1))
        x_pool = ctx.enter_context(tc.tile_pool(name="xin", bufs=x_bufs))
        e_pool = ctx.enter_context(tc.tile_pool(name="e", bufs=e_bufs))
        o_pool = ctx.enter_context(tc.tile_pool(name="o", bufs=o_bufs))
        s_pool = ctx.enter_context(tc.tile_pool(name="small", bufs=4))
        xt_pool = ctx.enter_context(tc.tile_pool(name="xt", bufs=xt_bufs, space="PSUM"))
        u_pool = ctx.enter_context(tc.tile_pool(name="u", bufs=u_bufs, space="PSUM"))

        # Constants go over SWDGE (Pool) so they don't delay the HWDGE x loads.
        ident_sb = const_pool.tile([BLK, BLK], F32)
        nc.gpsimd.dma_start(ident_sb[:], ident_ap[:, :])
        ident_bf = None
        xb_pool = None
        if bf16_t:
            ident_bf = const_pool.tile([BLK, BLK], BF16)
            nc.gpsimd.dma_start(ident_bf[:], ident_ap[:, :])
            xb_pool = ctx.enter_context(tc.tile_pool(name="xb", bufs=3))
        maug_sb = const_pool.tile([BLK, KCH * NAUG], BF16)
        nc.gpsimd.dma_start(
            maug_sb[:].rearrange("p (k n) -> p k n", n=NAUG),
            maug_ap[:, :, :].rearrange("k p n -> p k n"),
        )

        # bench_iters>0 wraps the whole body in an on-device loop so the
        # per-iteration time can be resolved through the ~1s axon RPC noise.
        # bench_iters < 0: staggered-reset back-edge (cross-iter overlap)
        loop_cm = (
            tc.For_i(0, abs(bench_iters), 1, staggered_reset=bench_iters < 0)
            if bench_iters
            else nullcontext()
        )
        ctx.enter_context(loop_cm)

        row0 = 0
        outs = None
        ob_blocks = 0       # blocks accumulated in current outs tile
        ob_row0 = 0         # first row-block covered by current outs tile
        OBW = out_batch * 2  # max blocks per outs tile

        def flush_outs():
            nonlocal outs, ob_blocks, ob_row0
            if outs is None or ob_blocks == 0:
                return
            # output DMA via SWDGE (Pool) — separate queue from the inputs
            if compute_only:
                outs = None
                ob_blocks = 0
                return
            out_eng = nc.sync if out_on_sync else nc.gpsimd
            out_eng.dma_start(
                out_ap[ob_row0 * BLK:(ob_row0 + ob_blocks) * BLK, :].rearrange(
                    "(b p) c -> p b c", p=BLK
                ),
                outs[:, 0:ob_blocks * P].rearrange("p (b c) -> p b c", c=P),
            )
            outs = None
            ob_blocks = 0

        for pn in PAIRS:
            # input DMA for this pair on the SP HWDGE ring (in-DMAs only, so
            # no out-DMA can head-of-line block the input stream)
            xs = x_pool.tile([BLK, 2 * C], F32, tag="xs")
            if compute_only:
                nc.vector.memset(xs[:, 0:8], 0.0)  # mark tile written
            else:
                nc.sync.dma_start(
                    xs[:, 0:pn * C].rearrange("p (b c) -> p b c", c=C),
                    x_ap[row0 * BLK:(row0 + pn) * BLK, :].rearrange(
                        "(b p) c -> p b c", p=BLK
                    ),
                )
            if outs is None:
                outs = o_pool.tile([BLK, OBW * P], F32, tag="outs")
                ob_row0 = row0
            if dma_only:
                if ob_blocks == 0:
                    nc.vector.memset(outs[:, 0:8], 0.0)  # mark tile written
                ob_blocks += pn
                row0 += pn
                if ob_blocks + 2 > OBW:
                    flush_outs()
                continue
            if bf16_t:
                # cast x to bf16 on an otherwise-idle engine; PE transposes
                # then run at 1 cycle/row (vs 2 for f32) and xt PSUM tiles
                # take 1 bank instead of 2
                xb = xb_pool.tile([BLK, 2 * C], BF16, tag="xb")
                getattr(nc, cast_eng).tensor_copy(xb[:, 0:pn * C], xs[:, 0:pn * C])
                t_src, t_ident, t_dt = xb, ident_bf, BF16
            else:
                t_src, t_ident, t_dt = xs, ident_sb, F32
            xt = xt_pool.tile([BLK, 2 * C], t_dt, tag="xt")  # PSUM
            for bb in range(pn):
                for k in range(KCH):
                    nc.tensor.transpose(
                        xt[:, bb * C + k * BLK:bb * C + (k + 1) * BLK],
                        t_src[:, bb * C + k * BLK:bb * C + (k + 1) * BLK],
                        t_ident[:],
                    )
            e = e_pool.tile([BLK, 2 * C], BF16, tag="e")  # exp(x), [c, r] layout
            nc.scalar.activation(
                e[:, 0:pn * C], xt[:, 0:pn * C],
                mybir.ActivationFunctionType.Exp,
            )
            for bb in range(pn):
                u = u_pool.tile([BLK, NAUG], F32, tag="u")
                for k in range(KCH):
                    nc.tensor.matmul(
                        u[:],
                        e[:, bb * C + k * BLK:bb * C + (k + 1) * BLK],
                        maug_sb[:, k * NAUG:(k + 1) * NAUG],
                        start=(k == 0),
                        stop=(k == KCH - 1),
                    )
                inv_s = s_pool.tile([BLK, 1], F32, tag="inv")
                nc.vector.reciprocal(inv_s[:], u[:, P + 1:P + 2])
                bias_t = s_pool.tile([BLK, 1], F32, tag="bias")
                # bias = (s/P - w/P) * (1/s) = (s - w)/(P*s)
                nc.vector.scalar_tensor_tensor(
                    bias_t[:],
                    u[:, P + 2:P + 3],
                    u[:, P:P + 1],
                    inv_s[:],
                    op0=mybir.AluOpType.subtract,
                    op1=mybir.AluOpType.mult,
                )
                ob = ob_blocks + bb
                nc.scalar.activation(
                    outs[:, ob * P:(ob + 1) * P],
                    u[:, 0:P],
                    mybir.ActivationFunctionType.Ln,
                    bias=bias_t[:],
                    scale=inv_s[:],
                )
            ob_blocks += pn
            row0 += pn
            if ob_blocks + 2 > OBW:
                flush_outs()
        flush_outs()
    try:
        nc.compile()
    finally:
        _restore_tabs()
    return nc


def build_graph_t(x_bufs=3, e_bufs=3, o_bufs=3, u_bufs=6, out_batch=1,
                  rs=512, in_split=2, exp_split=1, out_eng="sync",
                  out_blocks=64, exp_mode="k", maug_bf16=False, ln_batch=1,
                  x_bf16=False, out_bf16=False, fused_bias=False, rsched=None,
                  bench_iters=0, skip_mm=False, skip_dve=False):
    """Variant taking the per-core x shard PRE-TRANSPOSED on the host:
    x_t[C, R_PER_CORE].  No on-chip transposes: DMA loads [128c, r] tiles
    directly, exp runs on big tiles, PE does only the matmuls.

    fused_bias: the redistribution bias is baked into M on the host
    (M'[c,p] = M[c,p] + (1 - rowsum(M)[c])/P), so U' = exp(x) @ M'aug
    already holds sq + (s-w)/P and only cols [0:P] plus a ones column
    (s, at P) are needed: NAUG -> 257 and the per-block DVE work drops
    to reciprocal + one fused multiply-evacuate.
    """
    from contextlib import ExitStack, nullcontext

    naug = P + 1 if fused_bias else NAUG
    nc = bacc.Bacc("TRN2", target_bir_lowering=False, debug=False, num_devices=1)
    _restore_tabs = _pin_act_table(nc.m.arch)
    x_dt = BF16 if x_bf16 else F32
    x_ap = nc.dram_tensor("x", [C, R_PER_CORE], x_dt, kind="ExternalInput").ap()
    maug_dt = BF16 if maug_bf16 else F32
    maug_ap = nc.dram_tensor("maug", [KCH, BLK, naug], maug_dt, kind="ExternalInput").ap()
    out_dt = BF16 if out_bf16 else F32
    out_ap = nc.dram_tensor("out", [R_PER_CORE, P], out_dt, kind="ExternalOutput").ap()

    if rsched is None:
        rsched = [rs] * (R_PER_CORE // rs)
    assert sum(rsched) == R_PER_CORE

    with tile.TileContext(nc) as tc, ExitStack() as ctx:
        const_pool = ctx.enter_context(tc.tile_pool(name="const", bufs=1))
        x_pool = ctx.enter_context(tc.tile_pool(name="xin", bufs=x_bufs))
        e_pool = ctx.enter_context(tc.tile_pool(name="e", bufs=e_bufs))
        o_pool = ctx.enter_context(tc.tile_pool(name="o", bufs=o_bufs))
        s_pool = ctx.enter_context(tc.tile_pool(name="small", bufs=4))
        v_pool = ctx.enter_context(tc.tile_pool(name="v", bufs=3))
        u_pool = ctx.enter_context(tc.tile_pool(name="u", bufs=u_bufs, space="PSUM"))

        maug_sb = const_pool.tile([BLK, KCH * naug], BF16)
        nc.gpsimd.dma_start(
            maug_sb[:].rearrange("p (k n) -> p k n", n=naug),
            maug_ap[:, :, :].rearrange("k p n -> p k n"),
        )

        loop_cm = (
            tc.For_i(0, abs(bench_iters), 1) if bench_iters else nullcontext()
        )
        ctx.enter_context(loop_cm)

        x_t3 = x_ap.rearrange("(k p) r -> k p r", p=BLK)   # [KCH, 128, R]
        kper = KCH // in_split                              # c-chunks per in-DMA
        RSMAX = max(rsched)
        r0 = 0
        for rs in rsched:
            BPRS = rs // BLK
            # x slice [128, KCH*rs]: c-chunk k occupies cols [k*rs, (k+1)*rs)
            xs = x_pool.tile([BLK, KCH * RSMAX], x_dt, tag="xs")
            for d in range(in_split):
                nc.sync.dma_start(
                    xs[:, d * kper * rs:(d + 1) * kper * rs].rearrange(
                        "p (k r) -> p k r", r=rs
                    ),
                    x_t3[d * kper:(d + 1) * kper, :, r0:r0 + rs].rearrange(
                        "k p r -> p k r"
                    ),
                )
            e = e_pool.tile([BLK, KCH * RSMAX], BF16, tag="e")
            if exp_mode == "block":
                # one exp per row-block spanning all 4 c-chunks (strided AP):
                # each block's matmuls wait on ONE exp, not all of them
                x3 = xs[:, 0:KCH * rs].rearrange("p (k r) -> p k r", r=rs)
                e3 = e[:, 0:KCH * rs].rearrange("p (k r) -> p k r", r=rs)
                for b in range(rs // BLK):
                    nc.scalar.activation(
                        e3[:, :, b * BLK:(b + 1) * BLK],
                        x3[:, :, b * BLK:(b + 1) * BLK],
                        mybir.ActivationFunctionType.Exp,
                    )
            else:
                estep = KCH * rs // exp_split
                for s in range(exp_split):
                    nc.scalar.activation(
                        e[:, s * estep:(s + 1) * estep],
                        xs[:, s * estep:(s + 1) * estep],
                        mybir.ActivationFunctionType.Exp,
                    )
            outs = o_pool.tile([BLK, (RSMAX // BLK) * P], out_dt, tag="outs")
            if skip_mm:
                nc.vector.memset(outs[:, 0:8], 0.0)
            for b in range(BPRS if not skip_mm else 0):
                u = u_pool.tile([BLK, naug], F32, tag="u")
                for k in range(KCH):
                    nc.tensor.matmul(
                        u[:],
                        e[:, k * rs + b * BLK:k * rs + (b + 1) * BLK],
                        maug_sb[:, k * naug:(k + 1) * naug],
                        start=(k == 0),
                        stop=(k == KCH - 1),
                    )
                if fused_bias:
                    inv_s = s_pool.tile([BLK, 1], F32, tag="inv")
                    nc.vector.reciprocal(inv_s[:], u[:, P:P + 1])
                    if ln_batch > 1:
                        if b % ln_batch == 0:
                            v = v_pool.tile([BLK, ln_batch * P], F32, tag="v")
                        nc.vector.tensor_scalar(
                            v[:, (b % ln_batch) * P:(b % ln_batch + 1) * P],
                            u[:, 0:P],
                            inv_s[:],
                            None,
                            op0=mybir.AluOpType.mult,
                        )
                        if (b + 1) % ln_batch == 0 or b == BPRS - 1:
                            g0 = (b // ln_batch) * ln_batch
                            ng = b - g0 + 1
                            nc.scalar.activation(
                                outs[:, g0 * P:(g0 + ng) * P],
                                v[:, 0:ng * P],
                                mybir.ActivationFunctionType.Ln,
                            )
                    else:
                        nc.scalar.activation(
                            outs[:, b * P:(b + 1) * P],
                            u[:, 0:P],
                            mybir.ActivationFunctionType.Ln,
                            scale=inv_s[:],
                        )
                    if (b + 1) % out_blocks == 0 or b == BPRS - 1:
                        b0 = (b // out_blocks) * out_blocks
                        nb = b - b0 + 1
                        getattr(nc, out_eng).dma_start(
                            out_ap[r0 + b0 * BLK:r0 + (b0 + nb) * BLK, :].rearrange(
                                "(b p) c -> p b c", p=BLK
                            ),
                            outs[:, b0 * P:(b0 + nb) * P].rearrange(
                                "p (b c) -> p b c", c=P
                            ),
                        )
                    continue
                if skip_dve:
                    nc.scalar.activation(
                        outs[:, b * P:(b + 1) * P],
                        u[:, 0:P],
                        mybir.ActivationFunctionType.Ln,
                        bias=0.0,
                        scale=1.0,
                    )
                    continue
                inv_s = s_pool.tile([BLK, 1], F32, tag="inv")
                nc.vector.reciprocal(inv_s[:], u[:, P + 1:P + 2])
                bias_t = s_pool.tile([BLK, 1], F32, tag="bias")
                if ln_batch > 1:
                    # normalize on DVE (per-partition scalars), then one Ln
                    # per ln_batch blocks — fewer serial ACT instructions
                    if b % ln_batch == 0:
                        v = v_pool.tile([BLK, ln_batch * P], F32, tag="v")
                    # bias2 = s/P - w/P = (s - w)/P
                    nc.vector.tensor_scalar(
                        bias_t[:],
                        u[:, P + 2:P + 3],
                        u[:, P:P + 1],
                        None,
                        op0=mybir.AluOpType.subtract,
                    )
                    # V = (U + bias2) * inv_s
                    nc.vector.tensor_scalar(
                        v[:, (b % ln_batch) * P:(b % ln_batch + 1) * P],
                        u[:, 0:P],
                        bias_t[:],
                        inv_s[:],
                        op0=mybir.AluOpType.add,
                        op1=mybir.AluOpType.mult,
                    )
                    if (b + 1) % ln_batch == 0 or b == BPRS - 1:
                        g0 = (b // ln_batch) * ln_batch
                        ng = b - g0 + 1
                        nc.scalar.activation(
                            outs[:, g0 * P:(g0 + ng) * P],
                            v[:, 0:ng * P],
                            mybir.ActivationFunctionType.Ln,
                        )
                else:
                    # bias = (s/P - w/P) * (1/s) = (s - w)/(P*s)
                    nc.vector.scalar_tensor_tensor(
                        bias_t[:],
                        u[:, P + 2:P + 3],
                        u[:, P:P + 1],
                        inv_s[:],
                        op0=mybir.AluOpType.subtract,
                        op1=mybir.AluOpType.mult,
                    )
                    nc.scalar.activation(
                        outs[:, b * P:(b + 1) * P],
                        u[:, 0:P],
                        mybir.ActivationFunctionType.Ln,
                        bias=bias_t[:],
                        scale=inv_s[:],
                    )
                if (b + 1) % out_blocks == 0 or b == BPRS - 1:
                    b0 = (b // out_blocks) * out_blocks
                    nb = b - b0 + 1
                    getattr(nc, out_eng).dma_start(
                        out_ap[r0 + b0 * BLK:r0 + (b0 + nb) * BLK, :].rearrange(
                            "(b p) c -> p b c", p=BLK
                        ),
                        outs[:, b0 * P:(b0 + nb) * P].rearrange(
                            "p (b c) -> p b c", c=P
                        ),
                    )
            r0 += rs
    try:
        nc.compile()
    finally:
        _restore_tabs()
    return nc


def build_graph_t2(x_bufs=4, e_bufs=4, o_bufs=3, u_bufs=6, v_bufs=4, rs=512,
                   ln_batch=4, out_blocks=4, out_eng="sync", x_bf16=True,
                   out_bf16=True, x_fp8=False, sw_pipe=1, tiled_io=True,
                   hp_load=False, warm_mms=0, exp_split_r=1, rsched=None,
                   epi="ln", dve_exp=(), schr_sigma=4.0, s_eng="gpsimd",
                   ugrp=2, mm_fp8=False, unroll=1, body_warm=0, in_split2=0,
                   in_halves=False,
                   bench_iters=0, skip_exp=False, skip_mm=False, skip_in=False,
                   skip_out=False, skip_epi=False, skip_ln=False):
    """Software-pipelined fused-bias pipeline.

    Per tile: in-DMA -> exp -> per 128-row block: 4 matmuls (U' = e @ M'aug,
    NAUG=257) -> DVE reciprocal + multiply-evacuate -> batched ACT Ln ->
    out-DMA.  Emission order runs the DMA+exp of tile i+sw_pipe ahead of the
    block work of tile i so ACT never queues a Ln in front of a ready exp.

    tiled_io: host hands x pre-permuted to [NT, 128, KCH*rs] (per-partition
    contiguous) and takes out back as [NGRP, 128, out_blocks*P], so every
    DMA moves dense per-partition runs (4KB in / 2KB out) instead of 1KB/512B
    strided chunks.

    epi="fastlog": the whole per-block epilogue collapses to ONE DVE
    tensor_scalar using the bitcast-log identity
        ln(u/s) = (i32(u) - i32(s)) * ln2/2^23  + ln2*(eps_u - eps_s)
    (the 127-bias and the mean of the mantissa sawtooth eps cancel in the
    subtraction; residual |err| <= 0.06 ln-units, rel_l2 ~4e-3 vs 2e-2
    gate).  ACT then does exp only; DVE reads u [128,257] f32 PSUM once.

    dve_exp: tile indices whose exp runs on DVE instead of ACT via the
    Schraudolph bit-trick: i16 = x*(2^7/ln2) + (127*2^7 - sigma) written
    as int16 == bf16 bits of ~e^x.  bf16-in/int16-out single-src
    tensor_scalar hits the 4x DVE mode: ~(58+N/4)/0.96 ns vs ACT's
    (N+352)/1.2 -- 3.4x cheaper, used to balance ACT vs DVE.
    """
    from contextlib import ExitStack, nullcontext

    naug = P + 1
    nc = bacc.Bacc("TRN2", target_bir_lowering=False, debug=False, num_devices=1)
    _restore_tabs = _pin_act_table(nc.m.arch)
    x_dt = mybir.dt.float8e3 if x_fp8 else (BF16 if x_bf16 else F32)
    out_dt = BF16 if out_bf16 else F32

    if rsched is None:
        rsched = [rs] * (R_PER_CORE // rs)
    assert sum(rsched) == R_PER_CORE
    RSMAX = max(rsched)

    if tiled_io:
        # 2D tiled layouts: tile t occupies x cols [KCH*r0, KCH*(r0+rs));
        # block br occupies out cols [br*P, (br+1)*P).  Dense per-partition
        # runs on both sides, any rsched / flush grouping.
        x_ap = nc.dram_tensor("x", [BLK, KCH * R_PER_CORE], x_dt,
                              kind="ExternalInput").ap()
        out_ap = nc.dram_tensor("out", [BLK, (R_PER_CORE // BLK) * P], out_dt,
                                kind="ExternalOutput").ap()
    else:
        x_ap = nc.dram_tensor("x", [C, R_PER_CORE], x_dt, kind="ExternalInput").ap()
        out_ap = nc.dram_tensor("out", [R_PER_CORE, P], out_dt,
                                kind="ExternalOutput").ap()
    maug_ap = nc.dram_tensor("maug", [KCH, BLK, naug], F32, kind="ExternalInput").ap()

    with tile.TileContext(nc) as tc, ExitStack() as ctx:
        const_pool = ctx.enter_context(tc.tile_pool(name="const", bufs=1))
        x_pool = ctx.enter_context(tc.tile_pool(name="xin", bufs=x_bufs))
        e_pool = ctx.enter_context(tc.tile_pool(name="e", bufs=e_bufs))
        o_pool = ctx.enter_context(tc.tile_pool(name="o", bufs=o_bufs))
        s_pool = ctx.enter_context(tc.tile_pool(name="small", bufs=6))
        v_pool = ctx.enter_context(tc.tile_pool(name="v", bufs=v_bufs))
        if warm_mms or body_warm:
            u_bufs = min(u_bufs, 7)
            w_pool = ctx.enter_context(tc.tile_pool(name="warm", bufs=1,
                                                    space="PSUM"))
        u_pool = ctx.enter_context(tc.tile_pool(name="u", bufs=u_bufs, space="PSUM"))

        FP8 = mybir.dt.float8e4
        # mixed mode: DVE-Schraudolph tiles keep bf16 e (normal 4-MM blocks),
        # ACT-exp tiles use fp8 e + DoubleRow (2-MM blocks)
        tile_fp8 = (lambda ti: mm_fp8 and ti not in dve_exp)
        if mm_fp8:
            # fp8 maug, n-stride padded to 272 (DoubleRow pair-stride must be
            # a multiple of 16 bytes)
            naugp = 272
            maug_sb = const_pool.tile([BLK, KCH * naugp], FP8)
            nc.gpsimd.dma_start(
                maug_sb[:].rearrange("p (k n) -> p k n", n=naugp)[:, :, 0:naug],
                maug_ap[:, :, :].rearrange("k p n -> p k n"),
            )
        else:
            naugp = naug
            maug_sb = const_pool.tile([BLK, KCH * naug], BF16)
            nc.gpsimd.dma_start(
                maug_sb[:].rearrange("p (k n) -> p k n", n=naug),
                maug_ap[:, :, :].rearrange("k p n -> p k n"),
            )

        # unroll: emit the kernel body `unroll` times per For_i iteration so
        # back-to-back executions software-pipeline with no barrier between
        # them (tile pools cycle buffers across bodies) and the PE never
        # idles past the HAM MID window.  bench_iters counts TOTAL kernel
        # executions.
        unroll_eff = unroll
        assert abs(bench_iters) % unroll_eff == 0
        loop_cm = (
            tc.For_i(0, abs(bench_iters) // unroll_eff, 1,
                     staggered_reset=bench_iters < 0)
            if bench_iters else nullcontext()
        )
        ctx.enter_context(loop_cm)

        x_t3 = (None if tiled_io else
                x_ap.rearrange("(k p) r -> k p r", p=BLK))   # [KCH, 128, R]

        I16 = mybir.dt.int16
        I32 = mybir.dt.int32
        K_FLOG = float(np.log(2.0) / (1 << 23))
        C0_FLOG = float(127 * np.log(2.0) - 0.0298)
        A_SCHR = float((1 << 7) / np.log(2.0))
        B_SCHR = float(127 * (1 << 7) - schr_sigma)

        def load_tile(r0, rs, ti=-1):
            xs = x_pool.tile([BLK, KCH * RSMAX], x_dt, tag="xs")
            if skip_in:
                nc.vector.memset(xs[:, 0:8], 0.0)
            elif tiled_io:
                if in_halves:
                    # half-tile DMAs (k-chunks 0,1 then 2,3): exp of the
                    # first half starts while the second half streams,
                    # raising DMA-engine utilization at the roofline
                    h = KCH * rs // 2
                    nc.sync.dma_start(xs[:, 0:h],
                                      x_ap[:, KCH * r0:KCH * r0 + h])
                    nc.sync.dma_start(xs[:, h:KCH * rs],
                                      x_ap[:, KCH * r0 + h:KCH * (r0 + rs)])
                elif in_split2 and ti % 2 == 1:
                    # alternate tiles onto the SWDGE (gpsimd) queue so the
                    # two descriptor streams generate in parallel
                    nc.gpsimd.dma_start(xs[:, 0:KCH * rs],
                                        x_ap[:, KCH * r0:KCH * (r0 + rs)])
                else:
                    nc.sync.dma_start(xs[:, 0:KCH * rs],
                                      x_ap[:, KCH * r0:KCH * (r0 + rs)])
            else:
                nc.sync.dma_start(
                    xs[:, 0:KCH * rs].rearrange("p (k r) -> p k r", r=rs),
                    x_t3[:, :, r0:r0 + rs].rearrange("k p r -> p k r"),
                )
            e = e_pool.tile([BLK, KCH * RSMAX], FP8 if tile_fp8(ti) else BF16,
                            tag="e")
            if skip_exp:
                nc.vector.memset(e[:, 0:8], 0.0)
            elif ti in dve_exp:
                # Schraudolph exp on DVE: bf16 src -> int16 bits of bf16 e^x
                nc.vector.tensor_scalar(
                    e[:, 0:KCH * rs].bitcast(I16),
                    xs[:, 0:KCH * rs],
                    A_SCHR,
                    B_SCHR,
                    op0=mybir.AluOpType.mult,
                    op1=mybir.AluOpType.add,
                )
            elif exp_split_r > 1:
                # split exp along r (strided across the KCH chunks) so the
                # first blocks' matmuls release before the whole tile's exp
                x3 = xs[:, 0:KCH * rs].rearrange("p (k r) -> p k r", r=rs)
                e3 = e[:, 0:KCH * rs].rearrange("p (k r) -> p k r", r=rs)
                step = rs // exp_split_r
                for s in range(exp_split_r):
                    nc.scalar.activation(
                        e3[:, :, s * step:(s + 1) * step],
                        x3[:, :, s * step:(s + 1) * step],
                        mybir.ActivationFunctionType.Exp,
                    )
            elif in_halves:
                # exp per DMA half so the first half's exp overlaps the
                # second half's transfer
                h = KCH * rs // 2
                for c0, c1 in ((0, h), (h, KCH * rs)):
                    nc.scalar.activation(
                        e[:, c0:c1], xs[:, c0:c1],
                        mybir.ActivationFunctionType.Exp,
                    )
            else:
                nc.scalar.activation(
                    e[:, 0:KCH * rs], xs[:, 0:KCH * rs],
                    mybir.ActivationFunctionType.Exp,
                )
            return e

        def dma_out(outs, r0, b0, nb):
            if skip_out:
                return
            if tiled_io:
                br0 = r0 // BLK + b0
                getattr(nc, out_eng).dma_start(
                    out_ap[:, br0 * P:(br0 + nb) * P],
                    outs[:, b0 * P:(b0 + nb) * P]
                )
            else:
                getattr(nc, out_eng).dma_start(
                    out_ap[r0 + b0 * BLK:r0 + (b0 + nb) * BLK, :].rearrange(
                        "(b p) c -> p b c", p=BLK
                    ),
                    outs[:, b0 * P:(b0 + nb) * P].rearrange("p (b c) -> p b c", c=P),
                )

        def block_work_fastlog(e, r0, rs):
            # Blocks processed in groups of `ugrp` sharing one PSUM tile of
            # ugrp banks ([128, 512*ugrp] f32, matmul b at cols 512*b:+257 so
            # each MM output stays inside one 2KB bank).  One strided DVE
            # tensor_copy converts the group's i32(s) columns to f32, then
            # each block needs exactly one DVE tensor_scalar:
            # out = (i32(u) - f32(i32(s))) * ln2/2^23.
            BPRS = rs // BLK
            outs = o_pool.tile([BLK, (RSMAX // BLK) * P], out_dt, tag="outs")
            for pb in range((BPRS + ugrp - 1) // ugrp):
                nb = min(ugrp, BPRS - pb * ugrp)    # blocks in this group
                u2 = u_pool.tile([BLK, 512 * ugrp], F32, tag="u2")
                u3 = u2[:].rearrange("p (h c) -> p h c", c=512)
                for i in range(nb):
                    b = pb * ugrp + i
                    if e[:, 0:1].dtype == FP8:
                        # DoubleRow: contraction 256 per MM (2 c-chunks
                        # interleaved pairwise), 2 MMs per block instead of 4
                        e3 = e[:, 0:KCH * rs].rearrange("p (k r) -> p k r",
                                                        r=rs)
                        m3 = maug_sb[:].rearrange("p (k n) -> p k n", n=naugp)
                        for q in range(KCH // 2):
                            nc.tensor.matmul(
                                u3[:, i, 0:naug],
                                e3[:, 2 * q:2 * q + 2, b * BLK:(b + 1) * BLK],
                                m3[:, 2 * q:2 * q + 2, 0:naug],
                                start=(q == 0),
                                stop=(q == KCH // 2 - 1),
                                perf_mode=mybir.MatmulPerfMode.DoubleRow,
                            )
                        continue
                    for k in range(KCH):
                        nc.tensor.matmul(
                            u3[:, i, 0:naug],
                            e[:, k * rs + b * BLK:k * rs + (b + 1) * BLK],
                            maug_sb[:, k * naug:(k + 1) * naug],
                            start=(k == 0),
                            stop=(k == KCH - 1),
                        )
                if epi == "fastlog2":
                    # exact per-row ln(s) on ACT (tiny, from PSUM; natural_log
                    # table is resident anyway) + centering constant on
                    # GPSIMD: out = i32(u)*k - (127*ln2 - eps_c + ln(s)).
                    # Residual error is the one-sided mantissa sawtooth
                    # centered to +/-0.030 ln-units (vs +/-0.060 two-sided
                    # for the i32(s)-subtract variant).
                    lnp = s_pool.tile([BLK, ugrp], F32, tag="lnp")
                    for i in range(nb):
                        nc.scalar.activation(
                            lnp[:, i:i + 1], u3[:, i, P:P + 1],
                            mybir.ActivationFunctionType.Ln,
                        )
                    lnpc = s_pool.tile([BLK, ugrp], F32, tag="lnpc")
                    nc.gpsimd.tensor_scalar(
                        lnpc[:, 0:nb], lnp[:, 0:nb], C0_FLOG, None,
                        op0=mybir.AluOpType.add,
                    )
                else:
                    fs = s_pool.tile([BLK, ugrp], F32, tag="fs")
                    nc.vector.tensor_copy(fs[:, 0:nb],
                                          u3[:, 0:nb, P:P + 1].bitcast(I32))
                for i in range(nb):
                    b = pb * ugrp + i
                    if epi == "fastlog2":
                        nc.vector.tensor_scalar(
                            outs[:, b * P:(b + 1) * P],
                            u3[:, i, 0:P].bitcast(I32),
                            K_FLOG,
                            lnpc[:, i:i + 1],
                            op0=mybir.AluOpType.mult,
                            op1=mybir.AluOpType.subtract,
                        )
                    else:
                        nc.vector.tensor_scalar(
                            outs[:, b * P:(b + 1) * P],
                            u3[:, i, 0:P].bitcast(I32),
                            fs[:, i:i + 1],
                            K_FLOG,
                            op0=mybir.AluOpType.subtract,
                            op1=mybir.AluOpType.mult,
                        )
                    if (b + 1) % out_blocks == 0 or b == BPRS - 1:
                        b0 = (b // out_blocks) * out_blocks
                        dma_out(outs, r0, b0, b - b0 + 1)

        def block_work(e, r0, rs):
            if epi in ("fastlog", "fastlog2") and not (skip_mm or skip_epi):
                return block_work_fastlog(e, r0, rs)
            BPRS = rs // BLK
            outs = o_pool.tile([BLK, (RSMAX // BLK) * P], out_dt, tag="outs")
            if skip_mm:
                nc.vector.memset(outs[:, 0:8], 0.0)
            v = None
            for b in range(BPRS if not skip_mm else 0):
                u = u_pool.tile([BLK, naug], F32, tag="u")
                for k in range(KCH):
                    nc.tensor.matmul(
                        u[:],
                        e[:, k * rs + b * BLK:k * rs + (b + 1) * BLK],
                        maug_sb[:, k * naug:(k + 1) * naug],
                        start=(k == 0),
                        stop=(k == KCH - 1),
                    )
                if skip_epi:
                    if b == BPRS - 1:
                        nc.vector.memset(outs[:, 0:8], 0.0)
                        dma_out(outs, r0, 0, BPRS)
                    continue
                inv_s = s_pool.tile([BLK, 1], F32, tag="inv")
                nc.vector.reciprocal(inv_s[:], u[:, P:P + 1])
                if ln_batch > 1:
                    if b % ln_batch == 0:
                        v = v_pool.tile([BLK, min(ln_batch, BPRS) * P], F32, tag="v")
                    nc.vector.tensor_scalar(
                        v[:, (b % ln_batch) * P:(b % ln_batch + 1) * P],
                        u[:, 0:P],
                        inv_s[:],
                        None,
                        op0=mybir.AluOpType.mult,
                    )
                    if ((b + 1) % ln_batch == 0 or b == BPRS - 1) and not skip_ln:
                        g0 = (b // ln_batch) * ln_batch
                        ng = b - g0 + 1
                        nc.scalar.activation(
                            outs[:, g0 * P:(g0 + ng) * P],
                            v[:, 0:ng * P],
                            mybir.ActivationFunctionType.Ln,
                        )
                elif not skip_ln:
                    nc.scalar.activation(
                        outs[:, b * P:(b + 1) * P],
                        u[:, 0:P],
                        mybir.ActivationFunctionType.Ln,
                        scale=inv_s[:],
                    )
                if skip_ln and b == BPRS - 1:
                    nc.vector.memset(outs[:, 0:8], 0.0)
                if (b + 1) % out_blocks == 0 or b == BPRS - 1:
                    b0 = (b // out_blocks) * out_blocks
                    dma_out(outs, r0, b0, b - b0 + 1)
            if skip_mm:
                dma_out(outs, r0, 0, BPRS)

        from contextlib import nullcontext as _nullcm

        # HAM warm-keeper: PE idles through the loop barrier + first
        # DMA/exp fill (> the ~3.4us MID window), so without these the real
        # matmuls open at 1.2 GHz.  Dummy matmuls on resident constants have
        # no input deps and earliest priority -> they run during the fill,
        # keeping the PE activity window hot.
        for w in range(warm_mms):
            wt = w_pool.tile([BLK, naug], F32, tag="w")
            nc.tensor.matmul(
                wt[:], maug_sb[:, 0:BLK], maug_sb[:, 0:naug],
                start=True, stop=True,
            )

        # `pending` persists across unrolled bodies: body k+1's loads are
        # emitted before body k's last block_work, so the scheduler
        # interleaves cross-body DMA+exp ahead of trailing epilogues.
        pending = []
        for _body in range(unroll_eff):
            # per-body HAM warm-keepers: fill this body's PE idle window so
            # the clock-gate never drops to 4/8 between bodies
            for w in range(body_warm):
                wt = w_pool.tile([BLK, naug], F32, tag="w")
                nc.tensor.matmul(
                    wt[:], maug_sb[:, 0:BLK], maug_sb[:, 0:naug],
                    start=True, stop=True,
                )
            r0 = 0
            for ti, rs_i in enumerate(rsched):
                # hp_load biases the scheduler to front-load DMA+exp so ACT
                # never parks a Ln ahead of a ready exp
                with (tc.high_priority() if hp_load else _nullcm()):
                    e = load_tile(r0, rs_i, ti)
                pending.append((e, r0, rs_i))
                r0 += rs_i
                if len(pending) > sw_pipe:
                    block_work(*pending.pop(0))
        for args in pending:
            block_work(*args)
    try:
        nc.compile()
    finally:
        _restore_tabs()
    return nc


def make_maug(alloW, phone_arc_labels, phoneme_arc_labels, fused=False):
    alloW = np.asarray(alloW, dtype=np.float64).reshape(-1)
    phone = np.asarray(phone_arc_labels).astype(np.int64).reshape(-1)
    phoneme = np.asarray(phoneme_arc_labels).astype(np.int64).reshape(-1)
    M = np.zeros((C, P), dtype=np.float64)
    np.add.at(M, (phone, phoneme), np.exp(alloW))
    if fused:
        # bake the redistribution bias into M:  U' = e @ M'aug directly holds
        # sq + (s-w)/P in cols [0:P] and s in col P
        maug = np.empty((C, P + 1), dtype=np.float64)
        maug[:, :P] = M + ((1.0 - M.sum(axis=1)) / P)[:, None]
        maug[:, P] = 1.0
        return maug.astype(np.float32).reshape(KCH, BLK, P + 1)
    maug = np.empty((C, NAUG), dtype=np.float64)
    maug[:, :P] = M
    maug[:, P] = M.sum(axis=1) / P          # U[:,256] = w/P
    maug[:, P + 1] = 1.0                    # U[:,257] = s  (softmax denom)
    maug[:, P + 2] = 1.0 / P                # U[:,258] = s/P
    return maug.astype(np.float32).reshape(KCH, BLK, NAUG)


_NC = None

# Production config: bf16 input shard + bf16 output (host casts / upcasts),
# fused-bias maug, tiled host I/O layouts, fastlog DVE epilogue (ACT does
# exp only; dve_exp Schraudolph tiles available but not needed), split
# in/out HWDGE rings, HAM warm-keeper matmuls, deep tile-pool buffering.
CONFIG = dict(x_bf16=True, out_bf16=True, rs=512, ln_batch=4, out_blocks=4,
              out_eng="scalar", sw_pipe=1, tiled_io=True,
              epi="fastlog", dve_exp=(3,), ugrp=4, warm_mms=0, mm_fp8=False,
              in_halves=True,
              x_bufs=8, e_bufs=8, o_bufs=6, v_bufs=6, u_bufs=2)
# uniform tile schedule: with the unrolled software-pipelined bench loop the
# fill/drain ramp is amortized across bodies, so fewer/larger tiles win
# (fewer 625ns HWDGE descriptor gens + fewer ACT instruction overheads);
# tiles 0 and 3 exp on DVE via Schraudolph to balance ACT vs DVE
RSCHED = [512, 512, 512, 512]


def _get_nc():
    global _NC
    if _NC is None:
        _NC = build_graph_t2(**CONFIG, rsched=RSCHED)
    return _NC


def make_in_maps(hs_pad, alloW, phone_arc_labels, phoneme_arc_labels,
                 x_bf16=True, fused=True, tiled_io=True, rs=512, x_fp8=False,
                 rsched=None):
    import ml_dtypes

    hs = np.ascontiguousarray(np.asarray(hs_pad, dtype=np.float32)).reshape(ROWS, C)
    maug = make_maug(alloW, phone_arc_labels, phoneme_arc_labels, fused=fused)
    x_np_dt = (ml_dtypes.float8_e3m4 if x_fp8 else
               ml_dtypes.bfloat16 if x_bf16 else np.float32)
    if rsched is None:
        rsched = [rs] * (R_PER_CORE // rs)

    def shard(i):
        h = hs[i * R_PER_CORE:(i + 1) * R_PER_CORE]          # [R, C]
        if tiled_io:
            # x2 [128, KCH*R]: tile t (rows r0..r0+rs) occupies cols
            # [KCH*r0, KCH*(r0+rs)); within it partition p holds
            # x[r0+r, k*128+p] at col KCH*r0 + k*rs + r — per-partition
            # contiguous DMA runs
            chunks, r0 = [], 0
            for rs_t in rsched:
                c = h[r0:r0 + rs_t].reshape(rs_t, KCH, BLK).transpose(2, 1, 0)
                chunks.append(c.reshape(BLK, KCH * rs_t))
                r0 += rs_t
            return np.ascontiguousarray(
                np.concatenate(chunks, axis=1).astype(x_np_dt))
        return np.ascontiguousarray(h.T.astype(x_np_dt))

    # data-parallel shard over rows; shards handed to the device pre-transposed
    # so the contraction dim lands on SBUF partitions with no on-chip transpose
    return [{"x": shard(i), "maug": maug} for i in range(N_CORES)]


def untile_out(out_core, out_blocks=None):
    """[128, NB*P] -> [R_PER_CORE, P]: block br partition p is row br*128+p."""
    nb = R_PER_CORE // BLK
    o = out_core.reshape(BLK, nb, P).transpose(1, 0, 2)
    return o.reshape(R_PER_CORE, P)


def run(hs_pad, alloW, phone_arc_labels, phoneme_arc_labels, n_phonemes, trace=False):
    import time

    assert int(n_phonemes) == P
    in_maps = make_in_maps(hs_pad, alloW, phone_arc_labels, phoneme_arc_labels,
                           x_bf16=CONFIG["x_bf16"], fused=True,
                           tiled_io=CONFIG.get("tiled_io", True),
                           rs=CONFIG["rs"], x_fp8=CONFIG.get("x_fp8", False),
                           rsched=RSCHED)
    nc = _get_nc()
    last_err = None
    for attempt in range(7):
        try:
            res = bass_utils.run_bass_kernel_spmd(
                nc, in_maps, core_ids=list(range(N_CORES)), trace=trace
            )
            break
        except Exception as e:  # transient NRT exec-unit errors recover on retry
            last_err = e
            time.sleep(min(2.0 * (attempt + 1), 10.0))
    else:
        raise last_err
    if CONFIG.get("tiled_io", True):
        outs = [untile_out(np.asarray(res.results[i]["out"], dtype=np.float32),
                           CONFIG["out_blocks"]) for i in range(N_CORES)]
    else:
        outs = [np.asarray(res.results[i]["out"], dtype=np.float32)
                for i in range(N_CORES)]
    out = np.concatenate(outs, axis=0)
    return out.reshape(B, T, P), res


def kernel(hs_pad, alloW, phone_arc_labels, phoneme_arc_labels, n_phonemes):
    out, _ = run(hs_pad, alloW, phone_arc_labels, phoneme_arc_labels, n_phonemes)
    return out

